# revision 1
# baseline (speedup 1.0000x reference)
# Channel-Attention Module (CAM) kernel for Trainium2, 8 NeuronCores.
#
# reference:
#   a   = x.reshape(B, N, C)                 # B=16, N=64*64=4096, C=512
#   G   = einsum('bnc,bnd->bcd', a, a)       # [B, C, C]
#   att = softmax(G, axis=-1)
#   out = gamma * einsum('bnc,bcd->bnd', a, att) + x
#
# Sharding: data-parallel over batch, 2 samples per core.
#
# Numerics: a @ att is rewritten as a @ R + a with R = att - I.  The
# dominant identity contribution is applied as an f32 elementwise op, so
# only a @ R runs through the bf16 tensor engine.  Folding gamma in:
#   out = a @ (gamma*(att - I)) + (1 + gamma) * a
# Both big matmuls (G and a@R) run in bf16 on the PE at full rate; the
# softmax and all elementwise work is f32.

from contextlib import ExitStack

import numpy as np
import ml_dtypes

B = 16
HW_H = 64
HW_W = 64
N = HW_H * HW_W          # 4096 pixels per sample
C = 512                  # channels
NCORES = 8
SPC = B // NCORES        # samples per core
P = 128                  # partitions
MT = C // P              # 4 c-tiles
NT = N // P              # 32 n-tiles per sample
NG = NT // 4             # 8 groups of 4 n-tiles (one 1MB DMA each)

_CACHE = {}


def _build():
    import concourse.bacc as bacc
    import concourse.tile as tile
    import concourse.mybir as mybir

    fp32 = mybir.dt.float32
    bf16 = mybir.dt.bfloat16
    AX = mybir.AxisListType.X
    OP = mybir.AluOpType
    AF = mybir.ActivationFunctionType

    nc = bacc.Bacc(
        "TRN2",
        target_bir_lowering=False,
        debug=False,
        enable_asserts=False,
        num_devices=NCORES,
    )
    x_d = nc.dram_tensor("x", [SPC * N, C], fp32, kind="ExternalInput").ap()
    gvec_d = nc.dram_tensor("gvec", [P, 1], fp32, kind="ExternalInput").ap()
    gp1_d = nc.dram_tensor("gp1vec", [P, 1], fp32, kind="ExternalInput").ap()
    ident_d = nc.dram_tensor("ident", [P, P], bf16, kind="ExternalInput").ap()
    gi_d = nc.dram_tensor("gI", [P, P], fp32, kind="ExternalInput").ap()
    out_d = nc.dram_tensor("out", [SPC * N, C], fp32, kind="ExternalOutput").ap()

    with tile.TileContext(nc) as tc, ExitStack() as ctx:
        p_const = ctx.enter_context(tc.tile_pool(name="pconst", bufs=1))
        p_in = ctx.enter_context(tc.tile_pool(name="pin", bufs=3))
        p_a16 = ctx.enter_context(tc.tile_pool(name="pa16", bufs=1))
        p_aT = ctx.enter_context(tc.tile_pool(name="paT", bufs=1))
        p_z = ctx.enter_context(tc.tile_pool(name="pz", bufs=1))
        p_sm = ctx.enter_context(tc.tile_pool(name="psm", bufs=2))
        p_R = ctx.enter_context(tc.tile_pool(name="pR", bufs=1))
        p_st = ctx.enter_context(tc.tile_pool(name="pst", bufs=4))
        pp_g = ctx.enter_context(tc.tile_pool(name="ppg", bufs=2, space="PSUM"))
        pp_t = ctx.enter_context(tc.tile_pool(name="ppt", bufs=2, space="PSUM"))
        pp_o = ctx.enter_context(tc.tile_pool(name="ppo", bufs=3, space="PSUM"))

        gvec = p_const.tile([P, 1], fp32, name="gvec_sb")
        nc.sync.dma_start(out=gvec, in_=gvec_d)
        gp1 = p_const.tile([P, 1], fp32, name="gp1_sb")
        nc.sync.dma_start(out=gp1, in_=gp1_d)
        ident = p_const.tile([P, P], bf16, name="ident_sb")
        nc.sync.dma_start(out=ident, in_=ident_d)
        gI = p_const.tile([P, P], fp32, name="gI_sb")
        nc.sync.dma_start(out=gI, in_=gi_d)

        for s in range(SPC):
            # ---- Phase A: load x, cast to bf16, pre-scale (1+gamma)*a ----
            a16 = []
            zb = []
            for g in range(NG):
                xt = p_in.tile([P, 4, C], fp32, tag="xt", name=f"xt_{s}_{g}")
                src = x_d[s * N + g * 512 : s * N + (g + 1) * 512, :].rearrange(
                    "(u p) c -> p u c", p=P
                )
                nc.sync.dma_start(out=xt, in_=src)
                a = p_a16.tile([P, 4, C], bf16, tag=f"a16_{g}", name=f"a16_{s}_{g}")
                nc.scalar.activation(a, xt, AF.Copy)
                z = p_z.tile([P, 4, C], fp32, tag=f"z_{g}", name=f"z_{s}_{g}")
                nc.scalar.activation(z, xt, AF.Copy, scale=gp1)
                a16.append(a)
                zb.append(z)

            # ---- Phase B: G = a^T a (bf16), softmax rows, R = gamma*(att-I) ----
            Rb = []
            for m in range(MT):
                psg = pp_g.tile([P, C], fp32, tag="psg", name=f"psg_{s}_{m}")
                for k in range(NT):
                    g, u = divmod(k, 4)
                    nc.tensor.matmul(
                        psg,
                        a16[g][:, u, m * P : (m + 1) * P],
                        a16[g][:, u, :],
                        start=(k == 0),
                        stop=(k == NT - 1),
                    )
                nmax = p_st.tile([P, 1], fp32, tag="nmax", name=f"nmax_{s}_{m}")
                nc.vector.reduce_max(nmax, psg, axis=AX, negate=True)
                E = p_sm.tile([P, C], fp32, tag="E", name=f"E_{s}_{m}")
                nc.scalar.activation(E, psg, AF.Exp, bias=nmax, scale=1.0)
                ssum = p_st.tile([P, 1], fp32, tag="ssum", name=f"ssum_{s}_{m}")
                nc.vector.reduce_sum(ssum, E, axis=AX)
                rin = p_st.tile([P, 1], fp32, tag="rin", name=f"rin_{s}_{m}")
                nc.vector.reciprocal(rin, ssum)
                ag = p_sm.tile([P, C], fp32, tag="ag", name=f"ag_{s}_{m}")
                nc.vector.tensor_scalar(ag, E, rin, gvec, OP.mult, OP.mult)
                nc.vector.tensor_sub(
                    ag[:, m * P : (m + 1) * P], ag[:, m * P : (m + 1) * P], gI
                )
                r = p_R.tile([P, C], bf16, tag=f"R_{m}", name=f"R_{s}_{m}")
                nc.scalar.activation(r, ag, AF.Copy)
                Rb.append(r)

            # ---- Phase C: aT = a^T via PE (regular matmul against identity) ----
            aT = [
                p_aT.tile([P, N], bf16, tag=f"aT_{j}", name=f"aT_{s}_{j}")
                for j in range(MT)
            ]
            for j in range(MT):
                for g in range(NG):
                    pst = pp_t.tile([P, C], fp32, tag="pst", name=f"pst_{s}_{j}_{g}")
                    for u in range(4):
                        nc.tensor.matmul(
                            pst[:, u * P : (u + 1) * P],
                            a16[g][:, u, j * P : (j + 1) * P],
                            ident,
                            start=(u == 0),
                            stop=(u == 3),
                        )
                    nc.vector.tensor_copy(aT[j][:, g * 512 : (g + 1) * 512], pst)

            # ---- Phase D: tmp = a @ R (bf16); z = tmp + (1+gamma)*a; store ----
            for g in range(NG):
                for u in range(4):
                    i = g * 4 + u
                    po = pp_o.tile([P, C], fp32, tag="po", name=f"po_{s}_{i}")
                    for k in range(MT):
                        nc.tensor.matmul(
                            po,
                            aT[k][:, i * P : (i + 1) * P],
                            Rb[k],
                            start=(k == 0),
                            stop=(k == MT - 1),
                        )
                    nc.vector.tensor_add(zb[g][:, u, :], po, zb[g][:, u, :])
                dst = out_d[s * N + g * 512 : s * N + (g + 1) * 512, :].rearrange(
                    "(u p) c -> p u c", p=P
                )
                nc.sync.dma_start(out=dst, in_=zb[g])

    nc.compile()
    return nc


def _get_nc():
    if "nc" not in _CACHE:
        _CACHE["nc"] = _build()
    return _CACHE["nc"]


def _in_maps(x, gamma):
    x = np.asarray(x).astype(np.float32, copy=False)
    g = np.float32(np.asarray(gamma).reshape(-1)[0])
    xs = x.reshape(B, N, C)
    gvec = np.full((P, 1), g, np.float32)
    gp1 = np.full((P, 1), np.float32(1.0) + g, np.float32)
    ident = np.eye(P, dtype=ml_dtypes.bfloat16)
    gi = (g * np.eye(P)).astype(np.float32)
    maps = []
    for r in range(NCORES):
        shard = np.ascontiguousarray(xs[r * SPC : (r + 1) * SPC].reshape(SPC * N, C))
        maps.append(
            {"x": shard, "gvec": gvec, "gp1vec": gp1, "ident": ident, "gI": gi}
        )
    return maps


def _run(x, gamma, trace=False):
    from concourse import bass_utils

    nc = _get_nc()
    res = bass_utils.run_bass_kernel_spmd(
        nc, _in_maps(x, gamma), core_ids=list(range(NCORES)), trace=trace
    )
    out = np.concatenate(
        [res.results[r]["out"].reshape(SPC, N, C) for r in range(NCORES)], axis=0
    )
    return out.reshape(B, HW_H, HW_W, C).astype(np.float32, copy=False), res


def kernel(x, gamma):
    out, _ = _run(x, gamma, trace=False)
    return out
